# revision 2
# baseline (speedup 1.0000x reference)
"""Gaussian label-splat density kernel for Trainium2 (8 NeuronCores).

Math (matches the reference): for each batch b
    gx[n, w] = exp(-(w - lx[n])^2 / (2 sigma^2))   (normalized over w)
    gy[n, h] = exp(-(h - ly[n])^2 / (2 sigma^2))   (normalized over h)
    density[b, 0] = sum_n outer(gy[n], gx[n]) = gy.T @ gx    (K = 64 labels)

batch_images contributes only its shape, so the kernel never touches it.

Sharding: core c -> (batch b = c // 2, row half t = c % 2, h0 = 256 * t).
Each core builds its own gaussians from a 1 KB label packet and emits a
(256, 512) output tile as two 128x512 matmuls. No cross-core comms.

Everything lives on partitions 0..63 (label n on partition n). Both
normalizers (1/Zx * 1/Zy) fold into the small gy slice, so the raw gx tile
feeds the matmul directly. Label packet layout (built on host):
    col 0 = -lx, col 1 = -ly, col 2 = h0 - ly, col 3 = sigma
"""

import numpy as np

import concourse.bacc as bacc
import concourse.tile as tile
from concourse import mybir
from concourse.bass_utils import run_bass_kernel_spmd

B, NLAB, H, W = 4, 64, 512, 512
P = 128
HALF = H // 2  # output rows per core
N_CORES = 8
F32 = mybir.dt.float32

_CACHE: list = []


def _build():
    AF = mybir.ActivationFunctionType
    nc = bacc.Bacc("TRN2", debug=False, target_bir_lowering=False, num_devices=N_CORES)
    labels = nc.dram_tensor("labels", (NLAB, 4), F32, kind="ExternalInput").ap()
    out = nc.dram_tensor("out", (HALF, W), F32, kind="ExternalOutput").ap()

    with tile.TileContext(nc) as tc:
        with (
            tc.tile_pool(name="sb", bufs=1) as pool,
            tc.tile_pool(name="ob", bufs=2) as opool,
            tc.tile_pool(name="ps", bufs=2, space="PSUM") as psum,
        ):
            L = pool.tile([NLAB, 4], F32)
            nc.sync.dma_start(out=L, in_=labels)

            I = pool.tile([NLAB, W], F32)
            nc.gpsimd.iota(
                I,
                pattern=[[1, W]],
                base=0,
                channel_multiplier=0,
                allow_small_or_imprecise_dtypes=True,
            )

            # M = -1 / (2 sigma^2) per partition (kept off ACT so the first
            # Square isn't gated on it)
            s2 = pool.tile([NLAB, 1], F32)
            nc.vector.tensor_mul(s2, L[:, 3:4], L[:, 3:4])
            s2n = pool.tile([NLAB, 1], F32)
            nc.vector.tensor_scalar_mul(s2n, s2, -2.0)
            M = pool.tile([NLAB, 1], F32)
            nc.vector.reciprocal(M, s2n)

            # full-range x/y gaussians; Z* = per-row sums from the ACT
            # accumulator. Gy is only needed for its normalizer.
            SQx = pool.tile([NLAB, W], F32)
            nc.scalar.activation(SQx, I, AF.Square, bias=L[:, 0:1], scale=1.0)
            Gx = pool.tile([NLAB, W], F32)
            Zx = pool.tile([NLAB, 1], F32)
            nc.scalar.activation(Gx, SQx, AF.Exp, scale=M, accum_out=Zx)

            SQy = pool.tile([NLAB, W], F32)
            nc.scalar.activation(SQy, I, AF.Square, bias=L[:, 1:2], scale=1.0)
            Gy = pool.tile([NLAB, W], F32)
            Zy = pool.tile([NLAB, 1], F32)
            nc.scalar.activation(Gy, SQy, AF.Exp, scale=M, accum_out=Zy)

            # y gaussian over this core's 256 rows (labels pre-shifted by h0)
            SQs = pool.tile([NLAB, HALF], F32)
            nc.scalar.activation(
                SQs, I[:, 0:HALF], AF.Square, bias=L[:, 2:3], scale=1.0
            )
            Gs = pool.tile([NLAB, HALF], F32)
            nc.scalar.activation(Gs, SQs, AF.Exp, scale=M)

            Rx = pool.tile([NLAB, 1], F32)
            nc.vector.reciprocal(Rx, Zx)
            Ry = pool.tile([NLAB, 1], F32)
            nc.vector.reciprocal(Ry, Zy)
            Rxy = pool.tile([NLAB, 1], F32)
            nc.vector.tensor_mul(Rxy, Rx, Ry)

            GYn = pool.tile([NLAB, HALF], F32)
            nc.vector.tensor_scalar_mul(GYn, Gs, Rxy)

            for t in range(2):
                acc = psum.tile([P, W], F32)
                nc.tensor.matmul(
                    acc,
                    GYn[:, t * P : (t + 1) * P],
                    Gx,
                    start=True,
                    stop=True,
                )
                Ot = opool.tile([P, W], F32)
                if t == 0:
                    nc.vector.tensor_copy(Ot, acc)
                else:
                    nc.scalar.copy(Ot, acc)
                nc.sync.dma_start(out=out[t * P : (t + 1) * P, :], in_=Ot)

    nc.compile()
    return nc


def _in_maps(batch_labels: np.ndarray, sigma: float) -> list:
    maps = []
    for c in range(N_CORES):
        b, t = divmod(c, 2)
        h0 = t * HALF
        lx = batch_labels[b, :, 0]
        ly = batch_labels[b, :, 1]
        packed = np.empty((NLAB, 4), np.float32)
        packed[:, 0] = -lx
        packed[:, 1] = -ly
        packed[:, 2] = h0 - ly
        packed[:, 3] = sigma
        maps.append({"labels": packed})
    return maps


def _get_nc():
    if not _CACHE:
        _CACHE.append(_build())
    return _CACHE[0]


def _gather(results) -> np.ndarray:
    density = np.empty((B, 1, H, W), np.float32)
    for c in range(N_CORES):
        b, t = divmod(c, 2)
        density[b, 0, t * HALF : (t + 1) * HALF, :] = results[c]["out"]
    return density


def kernel(batch_images, batch_labels, sigma) -> np.ndarray:
    batch_labels = np.asarray(batch_labels, dtype=np.float32)
    sigma = float(np.asarray(sigma))
    nc = _get_nc()
    res = run_bass_kernel_spmd(
        nc, _in_maps(batch_labels, sigma), core_ids=list(range(N_CORES))
    )
    return _gather(res.results)


# revision 5
# speedup vs baseline: 1.1770x; 1.1770x over previous
"""Gaussian label-splat density kernel for Trainium2 (8 NeuronCores).

Math (matches the reference): for each batch b
    gx[n, w] = exp(-(w - lx[n])^2 / (2 sigma^2))   (normalized over w)
    gy[n, h] = exp(-(h - ly[n])^2 / (2 sigma^2))   (normalized over h)
    density[b, 0] = sum_n outer(gy[n], gx[n]) = gy.T @ gx    (K = 64 labels)

batch_images contributes only its shape, so the kernel never touches it.

Sharding: core c -> (batch b = c // 2, row half t = c % 2, h0 = 256 * t).
Each core builds its own gaussians from a 1 KB label packet and emits a
(256, 512) output tile as two 128x512 matmuls. No cross-core comms.

Everything lives on partitions 0..63 (label n on partition n; walrus
requires equal partition bases for multi-operand ops). The squared
distances run on the otherwise-idle Vector engine so ACT only does the
three Exps (+ row sums via its accumulator). Both normalizers
(1/Zx * 1/Zy) fold into the x profile; the y slice feeds the matmul raw.
Matmuls run in f32r (single PE pass instead of fp32's LOW+HIGH).

Label packet (built on host):
    col 0 = -lx, col 1 = -ly, col 2 = h0 - ly, col 3 = sigma
"""

import numpy as np

import concourse.bacc as bacc
import concourse.tile as tile
from concourse import mybir
from concourse.bass_utils import run_bass_kernel_spmd

B, NLAB, H, W = 4, 64, 512, 512
P = 128
HALF = H // 2  # output rows per core
N_CORES = 8
F32 = mybir.dt.float32
F32R = mybir.dt.float32r

_CACHE: list = []


def _build():
    AF = mybir.ActivationFunctionType
    nc = bacc.Bacc(
        "TRN2",
        debug=False,
        target_bir_lowering=False,
        num_devices=N_CORES,
        enable_partition_id=False,
    )
    labels = nc.dram_tensor("labels", (NLAB, 4), F32, kind="ExternalInput").ap()
    out = nc.dram_tensor("out", (HALF, W), F32, kind="ExternalOutput").ap()

    with tile.TileContext(nc) as tc:
        with (
            tc.tile_pool(name="sb", bufs=1) as pool,
            tc.tile_pool(name="ob", bufs=2) as opool,
            tc.tile_pool(name="ps", bufs=2, space="PSUM") as psum,
        ):
            # input-independent warm-up op so walrus's ACT_TABLE_LOAD lands
            # here and hides under the label DMA's completion latency
            warm = pool.tile([NLAB, 1], F32)
            nc.vector.memset(warm, 0.0)
            nc.scalar.activation(warm, warm, AF.Exp, scale=1.0)

            L = pool.tile([NLAB, 4], F32)
            nc.sync.dma_start(out=L, in_=labels)

            I = pool.tile([NLAB, W], F32)
            nc.gpsimd.iota(
                I,
                pattern=[[1, W]],
                base=0,
                channel_multiplier=0,
                allow_small_or_imprecise_dtypes=True,
            )

            # M = -1 / (2 sigma^2) per partition
            s2 = pool.tile([NLAB, 1], F32)
            nc.vector.tensor_mul(s2, L[:, 3:4], L[:, 3:4])
            s2n = pool.tile([NLAB, 1], F32)
            nc.vector.tensor_scalar_mul(s2n, s2, -2.0)
            M = pool.tile([NLAB, 1], F32)
            nc.vector.reciprocal(M, s2n)

            # squared distances on DVE (frees ACT for the exps)
            Dx = pool.tile([NLAB, W], F32)
            nc.vector.tensor_scalar_add(Dx, I, L[:, 0:1])
            SQx = pool.tile([NLAB, W], F32)
            nc.vector.tensor_mul(SQx, Dx, Dx)
            Dy = pool.tile([NLAB, W], F32)
            nc.vector.tensor_scalar_add(Dy, I, L[:, 1:2])
            SQy = pool.tile([NLAB, W], F32)
            nc.vector.tensor_mul(SQy, Dy, Dy)
            Ds = pool.tile([NLAB, HALF], F32)
            nc.vector.tensor_scalar_add(Ds, I[:, 0:HALF], L[:, 2:3])
            SQs = pool.tile([NLAB, HALF], F32)
            nc.vector.tensor_mul(SQs, Ds, Ds)

            # exps; Z* = per-row sums from the ACT accumulator
            Gx = pool.tile([NLAB, W], F32)
            Zx = pool.tile([NLAB, 1], F32)
            nc.scalar.activation(Gx, SQx, AF.Exp, scale=M, accum_out=Zx)
            Gy = pool.tile([NLAB, W], F32)
            Zy = pool.tile([NLAB, 1], F32)
            nc.scalar.activation(Gy, SQy, AF.Exp, scale=M, accum_out=Zy)
            Gs = pool.tile([NLAB, HALF], F32R)
            nc.scalar.activation(Gs, SQs, AF.Exp, scale=M)

            Rx = pool.tile([NLAB, 1], F32)
            nc.vector.reciprocal(Rx, Zx)
            Ry = pool.tile([NLAB, 1], F32)
            nc.vector.reciprocal(Ry, Zy)
            Rxy = pool.tile([NLAB, 1], F32)
            nc.vector.tensor_mul(Rxy, Rx, Ry)

            # both normalizers fold into the matmul rhs; lhsT = Gs raw
            Gxn = pool.tile([NLAB, W], F32R)
            nc.vector.tensor_scalar_mul(Gxn, Gx, Rxy)

            lhsT = Gs[:]
            rhs = Gxn[:]
            for t in range(2):
                acc = psum.tile([P, W], F32)
                nc.tensor.matmul(
                    acc,
                    lhsT[:, t * P : (t + 1) * P],
                    rhs,
                    start=True,
                    stop=True,
                )
                Ot = opool.tile([P, W], F32)
                if t == 0:
                    nc.vector.tensor_copy(Ot, acc)
                else:
                    nc.scalar.copy(Ot, acc)
                nc.sync.dma_start(out=out[t * P : (t + 1) * P, :], in_=Ot)

    nc.compile()
    return nc


def _in_maps(batch_labels: np.ndarray, sigma: float) -> list:
    maps = []
    for c in range(N_CORES):
        b, t = divmod(c, 2)
        h0 = t * HALF
        lx = batch_labels[b, :, 0]
        ly = batch_labels[b, :, 1]
        packed = np.empty((NLAB, 4), np.float32)
        packed[:, 0] = -lx
        packed[:, 1] = -ly
        packed[:, 2] = h0 - ly
        packed[:, 3] = sigma
        maps.append({"labels": packed})
    return maps


def _get_nc():
    if not _CACHE:
        _CACHE.append(_build())
    return _CACHE[0]


def _gather(results) -> np.ndarray:
    density = np.empty((B, 1, H, W), np.float32)
    for c in range(N_CORES):
        b, t = divmod(c, 2)
        density[b, 0, t * HALF : (t + 1) * HALF, :] = results[c]["out"]
    return density


def kernel(batch_images, batch_labels, sigma) -> np.ndarray:
    batch_labels = np.asarray(batch_labels, dtype=np.float32)
    sigma = float(np.asarray(sigma))
    nc = _get_nc()
    res = run_bass_kernel_spmd(
        nc, _in_maps(batch_labels, sigma), core_ids=list(range(N_CORES))
    )
    return _gather(res.results)


# revision 6
# speedup vs baseline: 1.2884x; 1.0946x over previous
"""Gaussian label-splat density kernel for Trainium2 (8 NeuronCores).

Math (matches the reference): for each batch b
    gx[n, w] = exp(-(w - lx[n])^2 / (2 sigma^2))   (normalized over w)
    gy[n, h] = exp(-(h - ly[n])^2 / (2 sigma^2))   (normalized over h)
    density[b, 0] = sum_n outer(gy[n], gx[n]) = gy.T @ gx    (K = 64 labels)

batch_images contributes only its shape, so the kernel never touches it.

Sharding: core c -> (batch b = c // 2, row half t = c % 2, h0 = 256 * t).
Each core builds its own gaussians from a 1 KB label packet and emits a
(256, 512) output tile as two 128x512 matmuls. No cross-core comms.

Everything lives on partitions 0..63 (label n on partition n; walrus
requires equal partition bases for multi-operand ops). Engine balance,
tuned from NTFF traces: ACT does the x square + the three exps (+ the y
row-sum via its accumulator); DVE does the y/slice squares, the x
row-sum, the normalizers. Both normalizers (1/Zx * 1/Zy) fold into the
small y-slice (lhsT); the raw x profile is the matmul rhs. Matmuls run
in f32r (single PE pass instead of fp32's LOW+HIGH). An
input-independent warm-up exp pulls the ~1.3us ACT table load into the
label-DMA wait window, and the second output DMA issues from the
Scalar engine's HWDGE ring so the two stores leave on parallel queues.

Label packet (built on host):
    col 0 = -lx, col 1 = -ly, col 2 = h0 - ly, col 3 = sigma
"""

import numpy as np

import concourse.bacc as bacc
import concourse.tile as tile
from concourse import mybir
from concourse.bass_utils import run_bass_kernel_spmd

B, NLAB, H, W = 4, 64, 512, 512
P = 128
HALF = H // 2  # output rows per core
N_CORES = 8
F32 = mybir.dt.float32
F32R = mybir.dt.float32r

_CACHE: list = []


def _build():
    AF = mybir.ActivationFunctionType
    AX = mybir.AxisListType
    nc = bacc.Bacc(
        "TRN2",
        debug=False,
        target_bir_lowering=False,
        num_devices=N_CORES,
        enable_partition_id=False,
    )
    labels = nc.dram_tensor("labels", (NLAB, 4), F32, kind="ExternalInput").ap()
    out = nc.dram_tensor("out", (HALF, W), F32, kind="ExternalOutput").ap()

    with tile.TileContext(nc) as tc:
        with (
            tc.tile_pool(name="sb", bufs=1) as pool,
            tc.tile_pool(name="ob", bufs=2) as opool,
            tc.tile_pool(name="ps", bufs=2, space="PSUM") as psum,
        ):
            # input-independent warm-up op so walrus's ACT_TABLE_LOAD lands
            # here and hides under the label DMA's completion latency
            warm = pool.tile([NLAB, 1], F32)
            nc.vector.memset(warm, 0.0)
            nc.scalar.activation(warm, warm, AF.Exp, scale=1.0)

            L = pool.tile([NLAB, 4], F32)
            nc.sync.dma_start(out=L, in_=labels)

            I = pool.tile([NLAB, W], F32)
            nc.gpsimd.iota(
                I,
                pattern=[[1, W]],
                base=0,
                channel_multiplier=0,
                allow_small_or_imprecise_dtypes=True,
            )

            # M = -1 / (2 sigma^2) per partition: (sigma * sigma) * -2, recip
            s2n = pool.tile([NLAB, 1], F32)
            nc.vector.tensor_scalar(
                s2n, L[:, 3:4], L[:, 3:4], -2.0,
                mybir.AluOpType.mult, mybir.AluOpType.mult,
            )
            M = pool.tile([NLAB, 1], F32)
            nc.vector.reciprocal(M, s2n)

            # x square on ACT, y/slice squares on DVE
            SQx = pool.tile([NLAB, W], F32)
            nc.scalar.activation(SQx, I, AF.Square, bias=L[:, 0:1], scale=1.0)
            Dy = pool.tile([NLAB, W], F32)
            nc.vector.tensor_scalar_add(Dy, I, L[:, 1:2])
            SQy = pool.tile([NLAB, W], F32)
            nc.vector.tensor_mul(SQy, Dy, Dy)

            # x exp (f32r for the matmul); Zx on DVE so ACT moves straight on
            Gx = pool.tile([NLAB, W], F32R)
            nc.scalar.activation(Gx, SQx, AF.Exp, scale=M)
            Zx = pool.tile([NLAB, 1], F32)
            nc.vector.reduce_sum(Zx, Gx, axis=AX.X)
            Rx = pool.tile([NLAB, 1], F32)
            nc.vector.reciprocal(Rx, Zx)

            # y exp; Zy via the ACT accumulator
            Gy = pool.tile([NLAB, W], F32)
            Zy = pool.tile([NLAB, 1], F32)
            nc.scalar.activation(Gy, SQy, AF.Exp, scale=M, accum_out=Zy)

            # y slice square (DVE) + exp (ACT)
            Ds = pool.tile([NLAB, HALF], F32)
            nc.vector.tensor_scalar_add(Ds, I[:, 0:HALF], L[:, 2:3])
            SQs = pool.tile([NLAB, HALF], F32)
            nc.vector.tensor_mul(SQs, Ds, Ds)
            Gs = pool.tile([NLAB, HALF], F32)
            nc.scalar.activation(Gs, SQs, AF.Exp, scale=M)

            Ry = pool.tile([NLAB, 1], F32)
            nc.vector.reciprocal(Ry, Zy)
            Rxy = pool.tile([NLAB, 1], F32)
            nc.vector.tensor_mul(Rxy, Rx, Ry)

            # both normalizers fold into the small lhsT; rhs = Gx raw
            GYn = pool.tile([NLAB, HALF], F32R)
            nc.vector.tensor_scalar_mul(GYn, Gs, Rxy)

            for t in range(2):
                acc = psum.tile([P, W], F32)
                nc.tensor.matmul(
                    acc,
                    GYn[:, t * P : (t + 1) * P],
                    Gx,
                    start=True,
                    stop=True,
                )
                Ot = opool.tile([P, W], F32)
                if t == 0:
                    nc.vector.tensor_copy(Ot, acc)
                    nc.sync.dma_start(out=out[t * P : (t + 1) * P, :], in_=Ot)
                else:
                    nc.scalar.copy(Ot, acc)
                    nc.scalar.dma_start(out=out[t * P : (t + 1) * P, :], in_=Ot)

    nc.compile()
    return nc


def _in_maps(batch_labels: np.ndarray, sigma: float) -> list:
    maps = []
    for c in range(N_CORES):
        b, t = divmod(c, 2)
        h0 = t * HALF
        lx = batch_labels[b, :, 0]
        ly = batch_labels[b, :, 1]
        packed = np.empty((NLAB, 4), np.float32)
        packed[:, 0] = -lx
        packed[:, 1] = -ly
        packed[:, 2] = h0 - ly
        packed[:, 3] = sigma
        maps.append({"labels": packed})
    return maps


def _get_nc():
    if not _CACHE:
        _CACHE.append(_build())
    return _CACHE[0]


def _gather(results) -> np.ndarray:
    density = np.empty((B, 1, H, W), np.float32)
    for c in range(N_CORES):
        b, t = divmod(c, 2)
        density[b, 0, t * HALF : (t + 1) * HALF, :] = results[c]["out"]
    return density


def kernel(batch_images, batch_labels, sigma) -> np.ndarray:
    batch_labels = np.asarray(batch_labels, dtype=np.float32)
    sigma = float(np.asarray(sigma))
    nc = _get_nc()
    res = run_bass_kernel_spmd(
        nc, _in_maps(batch_labels, sigma), core_ids=list(range(N_CORES))
    )
    return _gather(res.results)
